# revision 2
# baseline (speedup 1.0000x reference)
"""VQ codebook (vector-quantization) kernel for 8 Trainium2 NeuronCores.

Math (reference):
    flat [N,D], embedding [D,K];  d_nk = |f_n|^2 + |e_k|^2 - 2 f_n.e_k
    k*(n) = argmin_k d_nk  (first min)
    quantized = embedding.T[k*]            -> reshape to x
    commit   = 0.25 * mean(q - x)^2
    codebook = mean((q - x)^2)
    estimator = x + (q - x)

Device strategy (data-parallel over rows, 8 cores x 8192 rows):
    argmin_k d = argmax_k s,  s_nk = f_n.e_k - |e_k|^2/2
    The -|e|^2/2 bias is folded into the matmul as a 65th contraction row:
      lhsT = [x_shard.T ; ones] (65 x 8192),  rhs = [embedding ; -e2/2] (65 x 4096)
    Per 128-row tile the PE writes scores into PSUM [128, 4096] (two
    2048-halves, double-buffered across 8 banks).  DVE finds top-8 values
    (InstMax) then their indices (InstMaxIndex) per half; a small batched
    tail combines the two halves (ties -> lower index, matching jnp argmin).
    Host gathers codebook rows and computes estimator + loss.
"""

import os
import sys
import numpy as np

if "/opt/trn_rl_repo" not in sys.path:
    sys.path.insert(0, "/opt/trn_rl_repo")

N_CORES = 8
N_FULL = 65536          # total rows  (64*32*32)
D = 64                  # embed dim
K = 4096                # codebook entries
ROWS = N_FULL // N_CORES  # 8192 rows per core
TILES = ROWS // 128       # 64 row-tiles per core
HALF_K = K // 2           # 2048 scores per PSUM half

_CACHE = {}


def _build_program(reps: int = 1):
    import concourse.bass as bass
    import concourse.bacc as bacc
    import concourse.mybir as mybir
    import concourse.tile as tile

    f32 = mybir.dt.float32
    u32 = mybir.dt.uint32

    nc = bacc.Bacc("TRN2", target_bir_lowering=False, debug=False)

    xt_d = nc.dram_tensor("xt", [D + 1, ROWS], f32, kind="ExternalInput")
    ea_d = nc.dram_tensor("ea", [D + 1, K], f32, kind="ExternalInput")
    mx_d = nc.dram_tensor("mx", [128, TILES], f32, kind="ExternalOutput")
    ix_d = nc.dram_tensor("ix", [128, TILES], f32, kind="ExternalOutput")

    with tile.TileContext(nc) as tc:
        with (
            tc.tile_pool(name="persist", bufs=1) as pp,
            tc.tile_pool(name="psum", bufs=2, space="PSUM") as psp,
        ):
            xt = pp.tile([D + 1, ROWS], f32, tag="xt")
            ea = pp.tile([D + 1, K], f32, tag="ea")
            # top-8 values / indices per (tile, half)
            mvA = pp.tile([128, TILES, 8], f32, tag="mvA")
            mvB = pp.tile([128, TILES, 8], f32, tag="mvB")
            miA = pp.tile([128, TILES, 8], u32, tag="miA")
            miB = pp.tile([128, TILES, 8], u32, tag="miB")

            nc.sync.dma_start(xt[:], xt_d[:])
            nc.sync.dma_start(ea[:], ea_d[:])

            for _rep in range(reps):
                for t in range(TILES):
                    lhsT = xt[:, t * 128:(t + 1) * 128]
                    for h in range(2):
                        ps = psp.tile([128, HALF_K], f32, tag="ps")
                        for c in range(4):
                            k0 = h * HALF_K + c * 512
                            nc.tensor.matmul(
                                ps[:, c * 512:(c + 1) * 512],
                                lhsT,
                                ea[:, k0:k0 + 512],
                                start=True,
                                stop=True,
                            )
                        mv = (mvA if h == 0 else mvB)[:, t, :]
                        mi = (miA if h == 0 else miB)[:, t, :]
                        nc.vector.max(mv, ps[:])
                        nc.vector.max_index(mi, mv, ps[:])

            # ---- batched tail: combine halves (ties -> lower k) ----
            a = pp.tile([128, TILES], f32, tag="a")
            b = pp.tile([128, TILES], f32, tag="b")
            ia = pp.tile([128, TILES], f32, tag="ia")
            ib = pp.tile([128, TILES], f32, tag="ib")
            sel = pp.tile([128, TILES], f32, tag="sel")
            gmax = pp.tile([128, TILES], f32, tag="gmax")
            idx = pp.tile([128, TILES], f32, tag="idx")
            tmp = pp.tile([128, TILES], f32, tag="tmp")

            nc.vector.tensor_copy(a[:], mvA[:, :, 0])
            nc.vector.tensor_copy(b[:], mvB[:, :, 0])
            nc.vector.tensor_copy(ia[:], miA[:, :, 0])
            nc.vector.tensor_copy(ib[:], miB[:, :, 0])

            AluOp = mybir.AluOpType
            nc.vector.tensor_tensor(gmax[:], a[:], b[:], op=AluOp.max)
            # sel = 1.0 where b strictly wins
            nc.vector.tensor_tensor(sel[:], b[:], a[:], op=AluOp.is_gt)
            # idx = ia + sel * (ib + 2048 - ia)
            nc.vector.tensor_scalar_add(tmp[:], ib[:], float(HALF_K))
            nc.vector.tensor_tensor(tmp[:], tmp[:], ia[:], op=AluOp.subtract)
            nc.vector.tensor_tensor(tmp[:], tmp[:], sel[:], op=AluOp.mult)
            nc.vector.tensor_tensor(idx[:], ia[:], tmp[:], op=AluOp.add)

            nc.sync.dma_start(mx_d[:], gmax[:])
            nc.sync.dma_start(ix_d[:], idx[:])

    nc.compile()
    return nc


def _get_program(reps: int = 1):
    key = ("nc", reps)
    if key not in _CACHE:
        _CACHE[key] = _build_program(reps)
    return _CACHE[key]


def kernel(x: np.ndarray, embedding: np.ndarray):
    from concourse.bass_utils import run_bass_kernel_spmd

    x = np.asarray(x)
    embedding = np.asarray(embedding)
    shape = x.shape
    flat = np.ascontiguousarray(x.reshape(-1, D)).astype(np.float32, copy=False)

    e2 = np.sum(embedding.astype(np.float32) ** 2, axis=0, dtype=np.float32)
    ea = np.empty((D + 1, K), dtype=np.float32)
    ea[:D] = embedding
    ea[D] = -0.5 * e2

    in_maps = []
    for c in range(N_CORES):
        shard = flat[c * ROWS:(c + 1) * ROWS]          # [8192, 64]
        xt = np.empty((D + 1, ROWS), dtype=np.float32)
        xt[:D] = shard.T
        xt[D] = 1.0
        in_maps.append({"xt": xt, "ea": ea})

    nc = _get_program()
    res = run_bass_kernel_spmd(nc, in_maps, core_ids=list(range(N_CORES)))

    idx_parts = []
    for c in range(N_CORES):
        ixf = res.results[c]["ix"]                      # [128, TILES] float32
        # row r (within core) = t*128 + p  ->  value at ixf[p, t]
        idx_parts.append(ixf.T.reshape(-1))
    idx = np.concatenate(idx_parts).astype(np.int64)    # [65536]

    q = embedding.T[idx]                                # [N, D] float32
    qx = q - flat
    est = (flat + qx).reshape(shape)

    commit = np.float32(0.25) * np.float32(np.mean(qx, dtype=np.float64)) ** 2
    codebook = np.float32(np.mean(qx.astype(np.float64) ** 2))
    loss = np.float32(commit + codebook)
    return est, loss
